# revision 25
# baseline (speedup 1.0000x reference)
"""Causal self-attention (B=4,T=2048,C=2048,H=16,D=128) on 8 TRN2 NeuronCores.

Strategy: tensor-parallel over heads (2 heads/core) for QKV + attention.
Per-batch AllToAll redistributes y^T so each core holds all channels for
256 (b,t) rows per batch; the t-sharded output projection for batch b-1 is
interleaved as PE filler into batch b's attention groups, hiding both the
collective and the projection. bf16 matmuls, fp32 PSUM accumulation,
softmax without max-subtraction, causal masking via 0/1 tiles multiplied
post-exp on DVE, RoPE in-place via host-side weight-row permutation +
partition swap. All bulk loads are single multi-dim DMAs (AP.rearrange).
"""
import os
import sys

sys.path.insert(0, "/opt/trn_rl_repo")

import numpy as np
import ml_dtypes

B, T, C, H, D = 4, 2048, 2048, 16, 128
NCORES = 8
HPC = H // NCORES          # 2 heads per core
BT = B * T                 # 8192
BTPC = BT // NCORES        # 1024 (b,t) rows per core for out-proj
KT = C // 128              # 16 contraction tiles
SCALE = 1.0 / float(np.sqrt(D))
BF16 = ml_dtypes.bfloat16

LAST_EXEC_NS = None
_CACHE = {}


def _build_nc():
    from contextlib import ExitStack
    from concourse import bacc, tile, mybir

    bf = mybir.dt.bfloat16
    f32 = mybir.dt.float32
    mult = mybir.AluOpType.mult
    add = mybir.AluOpType.add
    Exp = mybir.ActivationFunctionType.Exp

    nc = bacc.Bacc("TRN2", target_bir_lowering=False, debug=False,
                   num_devices=NCORES)

    xT_d = nc.dram_tensor("xT", [C, BT], bf, kind="ExternalInput")
    wq_d = nc.dram_tensor("wqT", [C, HPC * D], bf, kind="ExternalInput")
    wk_d = nc.dram_tensor("wkT", [C, HPC * D], bf, kind="ExternalInput")
    wv_d = nc.dram_tensor("wvT", [C, HPC * D], bf, kind="ExternalInput")
    cos_d = nc.dram_tensor("ccT", [D, T], bf, kind="ExternalInput")
    sin_d = nc.dram_tensor("ssT", [D, T], bf, kind="ExternalInput")
    mask_d = nc.dram_tensor("masks", [128, 2048], bf, kind="ExternalInput")
    wp_d = nc.dram_tensor("wpT", [C, C], bf, kind="ExternalInput")
    out_d = nc.dram_tensor("outT", [C, BTPC], f32, kind="ExternalOutput")

    with tile.TileContext(nc) as tc:
        with tc.tile_pool(name="dram", bufs=1, space="DRAM") as dram:
            # per-batch A2A: block j of in = my 2 heads' y^T for core j's
            # 256 t of this batch; block i of out = core i's heads for my t.
            # batch 3 is split by head (l) so the first half overlaps the
            # second half of its attention.
            a2a_in = [dram.tile([2048, 256], bf, name=f"a2a_in{b}")
                      for b in range(B - 1)]
            a2a_out = [dram.tile([2048, 256], bf, name=f"a2a_out{b}")
                       for b in range(B - 1)]
            a2a3_in = [dram.tile([1024, 256], bf, name=f"a2a3_in{l}")
                       for l in range(HPC)]
            a2a3_out = [dram.tile([1024, 256], bf, name=f"a2a3_out{l}")
                        for l in range(HPC)]
            warm_in = dram.tile([128, 16], bf, name="warm_in")
            warm_out = dram.tile([128, 16], bf, name="warm_out")

            with ExitStack() as ab:
                const = ab.enter_context(tc.tile_pool(name="const", bufs=1))
                wpool = ab.enter_context(tc.tile_pool(name="w", bufs=1))
                xtq_pool = ab.enter_context(tc.tile_pool(name="xtq", bufs=2))
                qk_pool = ab.enter_context(tc.tile_pool(name="qk", bufs=4))
                sw_pool = ab.enter_context(tc.tile_pool(name="sw", bufs=2))
                v_pool = ab.enter_context(tc.tile_pool(name="v", bufs=25))
                ex_pool = ab.enter_context(tc.tile_pool(name="ex", bufs=3))
                acc_pool = ab.enter_context(tc.tile_pool(name="acc", bufs=2))
                accf_pool = ab.enter_context(tc.tile_pool(name="accf", bufs=3))
                rec_pool = ab.enter_context(tc.tile_pool(name="rec", bufs=2))
                yn_pool = ab.enter_context(tc.tile_pool(name="yn", bufs=3))
                y2_pool = ab.enter_context(tc.tile_pool(name="y2", bufs=2))
                ob_pool = ab.enter_context(tc.tile_pool(name="ob", bufs=2))
                ps_a = ab.enter_context(
                    tc.tile_pool(name="ps_a", bufs=2, space="PSUM"))
                ps_sc = ab.enter_context(
                    tc.tile_pool(name="ps_sc", bufs=2, space="PSUM"))
                ps_y = ab.enter_context(
                    tc.tile_pool(name="ps_y", bufs=2, space="PSUM"))

                # ---- weights first (first chains need wq/wk + x), then
                # constants (needed only from RoPE onward) ----
                xT_v = xT_d.ap().rearrange("(kk p) t -> p kk t", kk=KT)
                wp_loaded = [False]
                xtq_tiles = {}       # (b, tt4) -> xtq tile

                def emit_xtq(b, tt4):
                    if (b, tt4) in xtq_tiles:
                        return xtq_tiles[(b, tt4)]
                    t_ = xtq_pool.tile([128, KT, 512], bf,
                                       name=f"xtq_{b}_{tt4}", tag="xtq")
                    c0 = 2048 * b + 512 * tt4
                    nc.sync.dma_start(t_[:], xT_v[:, :, c0:c0 + 512])
                    xtq_tiles[(b, tt4)] = t_
                    return t_

                # w layout [128, kk, cols]: partition p of chunk kk = row
                # 128*kk + p of the [C, cols] DRAM tensor.
                wq_sb = wpool.tile([128, KT, HPC * D], bf, name="wq_sb")
                nc.sync.dma_start(
                    wq_sb[:], wq_d.ap().rearrange("(kk p) f -> p kk f", kk=KT))
                wk_sb = wpool.tile([128, KT, HPC * D], bf, name="wk_sb")
                nc.sync.dma_start(
                    wk_sb[:], wk_d.ap().rearrange("(kk p) f -> p kk f", kk=KT))
                emit_xtq(0, 0)
                wv_sb = wpool.tile([128, KT, HPC * D], bf, name="wv_sb")
                nc.sync.dma_start(
                    wv_sb[:], wv_d.ap().rearrange("(kk p) f -> p kk f", kk=KT))
                cos_sb = const.tile([D, T], bf, name="cos_sb")
                nc.sync.dma_start(cos_sb[:], cos_d.ap())
                sin_sb = const.tile([D, T], bf, name="sin_sb")
                nc.sync.dma_start(sin_sb[:], sin_d.ap())
                mask_sb = const.tile([128, 2048], bf, name="mask_sb")
                nc.sync.dma_start(mask_sb[:], mask_d.ap())
                ones_sb = const.tile([128, 128], bf, name="ones_sb")
                nc.vector.memset(ones_sb[:], 1.0)
                wp_sb = wpool.tile([128, KT, C], bf, name="wp_sb")

                # warm up the collectives machinery (first A2A otherwise
                # pays ~11us trigger delay + a slow first execution)
                nc.gpsimd.collective_compute(
                    "AllToAll", mybir.AluOpType.bypass,
                    replica_groups=[list(range(NCORES))],
                    ins=[warm_in.opt()], outs=[warm_out.opt()])

                # ---- per-chain emitters ----
                def emit_qk_chain(b, l, wsb, dst, tt4, xtq):
                    ps = ps_a.tile([128, 512], f32, name=f"qkps_{b}_{l}_{tt4}",
                                   tag="ps_a")
                    for kk in range(KT):
                        nc.tensor.matmul(
                            ps[:],
                            wsb[:, kk, 128 * l:128 * l + 128],
                            xtq[:, kk, :],
                            start=(kk == 0), stop=(kk == KT - 1))
                    nc.vector.tensor_copy(
                        dst[:, 512 * tt4:512 * tt4 + 512], ps[:])

                def emit_v_chain(b, tk, xtq):
                    s = tk % 4
                    ps = ps_a.tile([128, 512], f32, name=f"vps_{b}_{tk}",
                                   tag="ps_a")
                    for kk in range(KT):
                        nc.tensor.matmul(
                            ps[:, 0:HPC * D],
                            xtq[:, kk, 128 * s:128 * s + 128],
                            wv_sb[:, kk, :],
                            start=(kk == 0), stop=(kk == KT - 1))
                    v_ = v_pool.tile([128, HPC * D], bf, name=f"v_{b}_{tk}",
                                     tag="v")
                    nc.vector.tensor_copy(v_[:], ps[:, 0:HPC * D])
                    return v_

                def emit_outproj_chain(b, ff, y2):
                    ps = ps_a.tile([128, 512], f32, name=f"ops_{b}_{ff}",
                                   tag="ps_a")
                    for kk in range(KT):
                        nc.tensor.matmul(
                            ps[:, 0:256],
                            wp_sb[:, kk, 128 * ff:128 * ff + 128],
                            y2[:, kk, :],
                            start=(kk == 0), stop=(kk == KT - 1))
                    ob = ob_pool.tile([128, 256], f32, name=f"ob_{b}_{ff}",
                                      tag="ob")
                    nc.scalar.copy(ob[:], ps[:, 0:256])
                    nc.sync.dma_start(
                        out_d.ap()[128 * ff:128 * ff + 128,
                                   256 * b:256 * b + 256], ob[:])

                # state carried across batches
                qraw = {}   # (l, 'q'/'k') -> current batch [128, T] tile
                vt = {}     # (b, tk) -> v tile [128 t, 256 d]
                filler = []          # list of closures for PE filler work

                a2a_in_v = [a2a_in[b].rearrange(
                    "(blk row) t -> row blk t", blk=8) for b in range(B - 1)]
                a2a_out_v = [a2a_out[b].rearrange(
                    "(kk p) t -> p kk t", kk=KT) for b in range(B - 1)]
                a2a3_in_v = [a2a3_in[l].rearrange(
                    "(blk row) t -> row blk t", blk=8) for l in range(HPC)]
                a2a3_out_v = [a2a3_out[l].rearrange(
                    "(kk2 p) t -> p kk2 t", kk2=8) for l in range(HPC)]

                for b in range(B):
                    # ---------- pre-attention: QKV + RoPE ----------
                    for l in range(HPC):
                        for nm in ('q', 'k'):
                            qraw[(l, nm)] = qk_pool.tile(
                                [128, T], bf, name=f"{nm}raw_{b}_{l}",
                                tag="qk")
                    for tt4 in range(4):
                        xtq = emit_xtq(b, tt4)
                        for l in range(HPC):
                            emit_qk_chain(b, l, wq_sb, qraw[(l, 'q')], tt4,
                                          xtq)
                            emit_qk_chain(b, l, wk_sb, qraw[(l, 'k')], tt4,
                                          xtq)
                        for s in range(4):
                            tk = 4 * tt4 + s
                            if (b, tk) not in vt:
                                vt[(b, tk)] = emit_v_chain(b, tk, xtq)
                    if not wp_loaded[0]:
                        nc.sync.dma_start(
                            wp_sb[:],
                            wp_d.ap().rearrange("(kk p) f -> p kk f", kk=KT))
                        wp_loaded[0] = True

                    # RoPE in-place: raw = raw*cos + swap(raw)*sin
                    for l in range(HPC):
                        for nm in ('q', 'k'):
                            raw = qraw[(l, nm)]
                            sw = sw_pool.tile([128, T], bf,
                                              name=f"sw_{b}_{l}_{nm}",
                                              tag="sw")
                            nc.sync.dma_start(sw[0:64, :], raw[64:128, :])
                            nc.sync.dma_start(sw[64:128, :], raw[0:64, :])
                            nc.vector.tensor_tensor(
                                raw[:], raw[:], cos_sb[:], op=mult)
                            nc.vector.tensor_tensor(
                                sw[:], sw[:], sin_sb[:], op=mult)
                            nc.vector.tensor_tensor(
                                raw[:], raw[:], sw[:], op=add)

                    # PE filler work for the attention phase, in pop order:
                    # first v-chains of batch b+1 (no collective dependency
                    # -> can never stall the in-order PE queue), then the
                    # out-proj of batch b-1 (its y2 lands mid-attention at
                    # the latest, even when the collective runs slow).
                    filler = []

                    def mk_vfill(bn, tk):
                        def go():
                            vt[(bn, tk)] = emit_v_chain(
                                bn, tk, emit_xtq(bn, tk // 4))
                        return go

                    if b + 1 < B:
                        # prefetch next batch's first two x-quarters
                        emit_xtq(b + 1, 0)
                        emit_xtq(b + 1, 1)
                        filler += [mk_vfill(b + 1, tk) for tk in range(8)]
                    if b >= 1:
                        # y2 via the scalar DMA queue so it is not
                        # head-of-line blocked behind rope swaps / x loads
                        y2 = y2_pool.tile([128, KT, 256], bf,
                                          name=f"y2_{b - 1}", tag="y2")
                        nc.scalar.dma_start(y2[:], a2a_out_v[b - 1])
                        filler += [
                            (lambda bb, f, yy: lambda:
                             emit_outproj_chain(bb, f, yy))(b - 1, ff, y2)
                            for ff in range(KT)]

                    # ---------- attention (l-outer, jj-inner) ----------
                    fill_credit = [0.0]

                    def emit_av(l, jj, g, ex, yps, ng):
                        for i in range(2):
                            tk = 2 * g + i
                            nc.tensor.matmul(
                                yps[:],
                                vt[(b, tk)][:, 128 * l:128 * l + 128],
                                ex[:, 512 * i:512 * i + 512],
                                start=(g == 0 and i == 0),
                                stop=(g == ng - 1 and i == 1))

                    def pop_filler():
                        if filler:
                            filler.pop(0)()

                    for l in range(HPC):
                        for jj in range(4):
                            qrot = qraw[(l, 'q')]
                            krot = qraw[(l, 'k')]
                            ng = 2 * (jj + 1)
                            yps = ps_y.tile([128, 512], f32,
                                            name=f"yps_{b}_{l}_{jj}",
                                            tag="ps_y")
                            acc2 = acc_pool.tile([128, 1024], bf,
                                                 name=f"acc_{b}_{l}_{jj}",
                                                 tag="acc")
                            prev = None
                            for g in range(ng):
                                scps = ps_sc.tile(
                                    [128, 1024], f32,
                                    name=f"sc_{b}_{l}_{jj}_{g}", tag="ps_sc")
                                for i in range(2):
                                    tk = 2 * g + i
                                    nc.tensor.matmul(
                                        scps[:, 512 * i:512 * i + 512],
                                        krot[:, 128 * tk:128 * tk + 128],
                                        qrot[:, 512 * jj:512 * jj + 512],
                                        start=True, stop=True)
                                ex = ex_pool.tile(
                                    [128, 1024], bf,
                                    name=f"ex_{b}_{l}_{jj}_{g}", tag="ex")
                                nc.scalar.activation(
                                    ex[:], scps[:], Exp, scale=SCALE)
                                if g >= 2 * jj:   # diagonal pair: mask
                                    p = g - 2 * jj
                                    nc.vector.tensor_tensor(
                                        ex[:], ex[:],
                                        mask_sb[:, 1024 * p:1024 * p + 1024],
                                        op=mult)
                                if g == 0:
                                    nc.vector.tensor_copy(acc2[:], ex[:])
                                else:
                                    nc.vector.tensor_tensor(
                                        acc2[:], acc2[:], ex[:], op=add)
                                # software pipeline: AV lags scores by one
                                if prev is not None:
                                    emit_av(l, jj, prev[0], prev[1], yps, ng)
                                prev = (g, ex)
                                fill_credit[0] += 1.0
                                if fill_credit[0] >= 2.0:
                                    fill_credit[0] -= 2.0
                                    pop_filler()
                            emit_av(l, jj, prev[0], prev[1], yps, ng)

                            # normalization, off the PE critical path
                            accf = accf_pool.tile([128, 512], bf,
                                                  name=f"af_{b}_{l}_{jj}",
                                                  tag="accf")
                            nc.vector.tensor_tensor(
                                accf[:], acc2[:, 0:512], acc2[:, 512:1024],
                                op=add)
                            pop_filler()
                            sums = ps_a.tile([128, 512], f32,
                                             name=f"sums_{b}_{l}_{jj}",
                                             tag="ps_a")
                            nc.tensor.matmul(sums[:], ones_sb[:], accf[:],
                                             start=True, stop=True)
                            rec = rec_pool.tile([128, 512], f32,
                                                name=f"rec_{b}_{l}_{jj}",
                                                tag="rec")
                            nc.vector.reciprocal_approx_fast(rec[:], sums[:])
                            yn = yn_pool.tile([128, 512], bf,
                                              name=f"yn_{b}_{l}_{jj}",
                                              tag="yn")
                            nc.vector.tensor_tensor(
                                yn[:], yps[:], rec[:], op=mult)
                            # scatter to a2a_in: q-range 512*jj covers the
                            # two 256-col t-blocks 2jj, 2jj+1
                            if b < B - 1:
                                nc.sync.dma_start(
                                    a2a_in_v[b][128 * l:128 * l + 128,
                                                2 * jj:2 * jj + 2, :],
                                    yn[:].rearrange("p (i t) -> p i t", i=2))
                            else:
                                nc.sync.dma_start(
                                    a2a3_in_v[l][:, 2 * jj:2 * jj + 2, :],
                                    yn[:].rearrange("p (i t) -> p i t", i=2))
                        # batch 3: fire head l's half-A2A as soon as its
                        # attention is done, overlapping the other head
                        if b == B - 1:
                            nc.gpsimd.collective_compute(
                                "AllToAll", mybir.AluOpType.bypass,
                                replica_groups=[list(range(NCORES))],
                                ins=[a2a3_in[l].opt()],
                                outs=[a2a3_out[l].opt()])


                    # ---------- collective for this batch ----------
                    if b < B - 1:
                        nc.gpsimd.collective_compute(
                            "AllToAll",
                            mybir.AluOpType.bypass,
                            replica_groups=[list(range(NCORES))],
                            ins=[a2a_in[b].opt()],
                            outs=[a2a_out[b].opt()],
                        )
                    # leftover fillers (rare): emit now
                    while filler:
                        filler.pop(0)()

                # ---------- drain: out-proj for last batch ----------
                # head l's half-A2A delivers the even (l=0) / odd (l=1) kk
                # chunks of y^T. Pass A (even kk, complete per-ff chains)
                # overlaps the second collective; partials park as bf16 in
                # the now-dead qk tiles. Pass B adds the odd kk.
                y2h0 = y2_pool.tile([128, 8, 256], bf, name="y23_0",
                                    tag="y2")
                nc.scalar.dma_start(y2h0[:], a2a3_out_v[0])
                y2h1 = y2_pool.tile([128, 8, 256], bf, name="y23_1",
                                    tag="y2")
                nc.scalar.dma_start(y2h1[:], a2a3_out_v[1])
                part = [qk_pool.tile([128, T], bf, name=f"part_{i}",
                                     tag="qk") for i in range(2)]
                for ff in range(KT):          # pass A: even kk
                    ps = ps_a.tile([128, 512], f32, name=f"dpsA_{ff}",
                                   tag="ps_a")
                    for i8 in range(8):
                        nc.tensor.matmul(
                            ps[:, 0:256],
                            wp_sb[:, 2 * i8, 128 * ff:128 * ff + 128],
                            y2h0[:, i8, :],
                            start=(i8 == 0), stop=(i8 == 7))
                    nc.vector.tensor_copy(
                        part[ff // 8][:, 256 * (ff % 8):256 * (ff % 8) + 256],
                        ps[:, 0:256])
                for ff in range(KT):          # pass B: odd kk + partial
                    ps = ps_a.tile([128, 512], f32, name=f"dpsB_{ff}",
                                   tag="ps_a")
                    for i8 in range(8):
                        nc.tensor.matmul(
                            ps[:, 0:256],
                            wp_sb[:, 2 * i8 + 1, 128 * ff:128 * ff + 128],
                            y2h1[:, i8, :],
                            start=(i8 == 0), stop=(i8 == 7))
                    ob = ob_pool.tile([128, 256], f32, name=f"ob3_{ff}",
                                      tag="ob")
                    nc.vector.tensor_tensor(
                        ob[:], ps[:, 0:256],
                        part[ff // 8][:, 256 * (ff % 8):256 * (ff % 8) + 256],
                        op=add)
                    nc.sync.dma_start(
                        out_d.ap()[128 * ff:128 * ff + 128,
                                   256 * (B - 1):256 * (B - 1) + 256],
                        ob[:])

    nc.compile()
    return nc


def _prep_inputs(x, rope_freqs, W_attn, W_proj):
    x = np.asarray(x, np.float32)
    rope_freqs = np.asarray(rope_freqs, np.float32)
    W_attn = np.asarray(W_attn, np.float32)
    W_proj = np.asarray(W_proj, np.float32)

    xT = np.ascontiguousarray(x.reshape(BT, C).T).astype(BF16)
    perm = np.concatenate([np.arange(0, D, 2), np.arange(1, D, 2)])
    theta = np.outer(rope_freqs.astype(np.float64), np.arange(T))
    cos_, sin_ = np.cos(theta), np.sin(theta)
    ccT = np.concatenate([cos_, cos_], axis=0).astype(BF16)   # (128, T)
    ssT = np.concatenate([-sin_, sin_], axis=0).astype(BF16)  # (128, T)
    masks = np.zeros((128, 2048), np.float32)
    for i in range(4):
        masks[:, 512 * i:512 * (i + 1)] = (
            np.arange(512)[None, :] >= (np.arange(128)[:, None] + 128 * i))
    masks = masks.astype(BF16)
    wpT = np.ascontiguousarray(W_proj.T).astype(BF16)

    in_maps = []
    for r in range(NCORES):
        wq_rows, wk_rows, wv_rows = [], [], []
        for l in range(HPC):
            h = HPC * r + l
            wq_rows.append(W_attn[D * h:D * h + D][perm])
            wk_rows.append(W_attn[C + D * h:C + D * h + D][perm])
            wv_rows.append(W_attn[2 * C + D * h:2 * C + D * h + D])
        in_maps.append({
            "xT": xT,
            "wqT": np.ascontiguousarray(
                np.concatenate(wq_rows, 0).T).astype(BF16),
            "wkT": np.ascontiguousarray(
                np.concatenate(wk_rows, 0).T).astype(BF16),
            "wvT": np.ascontiguousarray(
                np.concatenate(wv_rows, 0).T).astype(BF16),
            "ccT": ccT,
            "ssT": ssT,
            "masks": masks,
            "wpT": wpT,
        })
    return in_maps


def _ensure_trace_support():
    """Register the axon NTFF profiling hook if the image's antenv lacks it,
    and stub out the artifact upload (no bucket access in-container)."""
    import types
    import sys as _sys
    import antenv

    if "antenv.axon_hooks" not in _sys.modules:
        try:
            import antenv.axon_hooks  # noqa: F401
        except ImportError:
            mod = types.ModuleType("antenv.axon_hooks")
            _holder = {}
            mod.set_axon_ntff_profile_hook = (
                lambda h: _holder.__setitem__("h", h))
            mod.get_axon_ntff_profile_hook = lambda: _holder.get("h")
            _sys.modules["antenv.axon_hooks"] = mod
            antenv.axon_hooks = mod
    import antenv.axon_hooks as ah

    if ah.get_axon_ntff_profile_hook() is None:
        try:
            from trn_agent_boot.trn_boot import _ntff_profile_via_ctypes
            hook = _ntff_profile_via_ctypes("/opt/axon/libaxon_pjrt.so")
            if hook is not None:
                ah.set_axon_ntff_profile_hook(hook)
        except Exception as e:  # profiling stays off; run still works
            print(f"ntff hook registration failed: {e}", file=sys.stderr)
    from concourse import bass_utils as bu
    bu.upload_artifacts = lambda tmpdir: f"local://{tmpdir}"


def kernel(x, rope_freqs, W_attn, W_proj):
    global LAST_EXEC_NS
    from concourse import bass_utils

    if "nc" not in _CACHE:
        _CACHE["nc"] = _build_nc()
    nc = _CACHE["nc"]

    in_maps = _prep_inputs(x, rope_freqs, W_attn, W_proj)
    trace = os.environ.get("KERNEL_TRACE", "0") == "1"
    tmpdir = None
    if trace:
        _ensure_trace_support()
        tmpdir = os.environ.get("KERNEL_TRACE_DIR") or None
    res = bass_utils.run_bass_kernel_spmd(
        nc, in_maps, core_ids=list(range(NCORES)), trace=trace,
        tmpdir=tmpdir)
    LAST_EXEC_NS = res.exec_time_ns

    # core r's outT: [2048 chan, 4 batches x 256 t]; batch b chunk holds
    # global rows 2048*b + 256*r ... + 256
    out = np.empty((BT, C), np.float32)
    for r in range(NCORES):
        outT = np.asarray(res.results[r]["outT"], np.float32)
        for b in range(B):
            out[2048 * b + 256 * r:2048 * b + 256 * r + 256, :] = \
                outT[:, 256 * b:256 * b + 256].T
    return np.ascontiguousarray(out).reshape(B, T, C)


# revision 29
# speedup vs baseline: 1.0671x; 1.0671x over previous
"""Causal self-attention (B=4,T=2048,C=2048,H=16,D=128) on 8 TRN2 NeuronCores.

Strategy: tensor-parallel over heads (2 heads/core) for QKV + attention.
Per-batch AllToAll redistributes y^T so each core holds all channels for
256 (b,t) rows per batch; the t-sharded output projection for batch b-1 is
interleaved as PE filler into batch b's attention groups, hiding both the
collective and the projection. bf16 matmuls, fp32 PSUM accumulation,
softmax without max-subtraction, causal masking via 0/1 tiles multiplied
post-exp on DVE, RoPE in-place via host-side weight-row permutation +
partition swap. All bulk loads are single multi-dim DMAs (AP.rearrange).
"""
import os
import sys

sys.path.insert(0, "/opt/trn_rl_repo")

import numpy as np
import ml_dtypes

B, T, C, H, D = 4, 2048, 2048, 16, 128
NCORES = 8
HPC = H // NCORES          # 2 heads per core
BT = B * T                 # 8192
BTPC = BT // NCORES        # 1024 (b,t) rows per core for out-proj
KT = C // 128              # 16 contraction tiles
SCALE = 1.0 / float(np.sqrt(D))
BF16 = ml_dtypes.bfloat16

LAST_EXEC_NS = None
_CACHE = {}


def _build_nc():
    from contextlib import ExitStack
    from concourse import bacc, tile, mybir

    bf = mybir.dt.bfloat16
    f32 = mybir.dt.float32
    mult = mybir.AluOpType.mult
    add = mybir.AluOpType.add
    Exp = mybir.ActivationFunctionType.Exp

    nc = bacc.Bacc("TRN2", target_bir_lowering=False, debug=False,
                   num_devices=NCORES)

    xT_d = nc.dram_tensor("xT", [C, BT], bf, kind="ExternalInput")
    wq_d = nc.dram_tensor("wqT", [C, HPC * D], bf, kind="ExternalInput")
    wk_d = nc.dram_tensor("wkT", [C, HPC * D], bf, kind="ExternalInput")
    wv_d = nc.dram_tensor("wvT", [C, HPC * D], bf, kind="ExternalInput")
    cos_d = nc.dram_tensor("ccT", [D, T], bf, kind="ExternalInput")
    sin_d = nc.dram_tensor("ssT", [D, T], bf, kind="ExternalInput")
    mask_d = nc.dram_tensor("masks", [128, 2048], bf, kind="ExternalInput")
    wp_d = nc.dram_tensor("wpT", [C, C], bf, kind="ExternalInput")
    out_d = nc.dram_tensor("outT", [C, BTPC], f32, kind="ExternalOutput")

    with tile.TileContext(nc) as tc:
        with tc.tile_pool(name="dram", bufs=1, space="DRAM") as dram:
            # per-batch A2A: block j of in = my 2 heads' y^T for core j's
            # 256 t of this batch; block i of out = core i's heads for my t.
            # batch 3 is split by head (l) so the first half overlaps the
            # second half of its attention.
            a2a_in = [dram.tile([2048, 256], bf, name=f"a2a_in{b}")
                      for b in range(B - 1)]
            a2a_out = [dram.tile([2048, 256], bf, name=f"a2a_out{b}")
                       for b in range(B - 1)]
            a2a3_in = [dram.tile([1024, 256], bf, name=f"a2a3_in{l}")
                       for l in range(HPC)]
            a2a3_out = [dram.tile([1024, 256], bf, name=f"a2a3_out{l}")
                        for l in range(HPC)]
            warm_in = dram.tile([128, 16], bf, name="warm_in")
            warm_out = dram.tile([128, 16], bf, name="warm_out")

            with ExitStack() as ab:
                const = ab.enter_context(tc.tile_pool(name="const", bufs=1))
                wpool = ab.enter_context(tc.tile_pool(name="w", bufs=1))
                xtq_pool = ab.enter_context(tc.tile_pool(name="xtq", bufs=2))
                qk_pool = ab.enter_context(tc.tile_pool(name="qk", bufs=4))
                sw_pool = ab.enter_context(tc.tile_pool(name="sw", bufs=2))
                v_pool = ab.enter_context(tc.tile_pool(name="v", bufs=25))
                ex_pool = ab.enter_context(tc.tile_pool(name="ex", bufs=3))
                acc_pool = ab.enter_context(tc.tile_pool(name="acc", bufs=2))
                accf_pool = ab.enter_context(tc.tile_pool(name="accf", bufs=3))
                rec_pool = ab.enter_context(tc.tile_pool(name="rec", bufs=2))
                yn_pool = ab.enter_context(tc.tile_pool(name="yn", bufs=3))
                y2_pool = ab.enter_context(tc.tile_pool(name="y2", bufs=2))
                ob_pool = ab.enter_context(tc.tile_pool(name="ob", bufs=2))
                ps_a = ab.enter_context(
                    tc.tile_pool(name="ps_a", bufs=2, space="PSUM"))
                ps_sc = ab.enter_context(
                    tc.tile_pool(name="ps_sc", bufs=2, space="PSUM"))
                ps_y = ab.enter_context(
                    tc.tile_pool(name="ps_y", bufs=2, space="PSUM"))

                # ---- weights first (first chains need wq/wk + x), then
                # constants (needed only from RoPE onward) ----
                xT_v = xT_d.ap().rearrange("(kk p) t -> p kk t", kk=KT)
                wp_loaded = [False]
                xtq_tiles = {}       # (b, tt4) -> xtq tile

                def emit_xtq(b, tt4):
                    if (b, tt4) in xtq_tiles:
                        return xtq_tiles[(b, tt4)]
                    t_ = xtq_pool.tile([128, KT, 512], bf,
                                       name=f"xtq_{b}_{tt4}", tag="xtq")
                    c0 = 2048 * b + 512 * tt4
                    nc.sync.dma_start(t_[:], xT_v[:, :, c0:c0 + 512])
                    xtq_tiles[(b, tt4)] = t_
                    return t_

                # w layout [128, kk, cols]: partition p of chunk kk = row
                # 128*kk + p of the [C, cols] DRAM tensor.
                wq_sb = wpool.tile([128, KT, HPC * D], bf, name="wq_sb")
                nc.sync.dma_start(
                    wq_sb[:], wq_d.ap().rearrange("(kk p) f -> p kk f", kk=KT))
                emit_xtq(0, 0)
                wk_sb = wpool.tile([128, KT, HPC * D], bf, name="wk_sb")
                nc.sync.dma_start(
                    wk_sb[:], wk_d.ap().rearrange("(kk p) f -> p kk f", kk=KT))
                wv_sb = wpool.tile([128, KT, HPC * D], bf, name="wv_sb")
                nc.sync.dma_start(
                    wv_sb[:], wv_d.ap().rearrange("(kk p) f -> p kk f", kk=KT))
                cos_sb = const.tile([D, T], bf, name="cos_sb")
                nc.sync.dma_start(cos_sb[:], cos_d.ap())
                sin_sb = const.tile([D, T], bf, name="sin_sb")
                nc.sync.dma_start(sin_sb[:], sin_d.ap())
                mask_sb = const.tile([128, 2048], bf, name="mask_sb")
                nc.sync.dma_start(mask_sb[:], mask_d.ap())
                ones_sb = const.tile([128, 128], bf, name="ones_sb")
                nc.vector.memset(ones_sb[:], 1.0)
                wp_sb = wpool.tile([128, KT, C], bf, name="wp_sb")

                # warm up the collectives machinery (first A2A otherwise
                # pays ~11us trigger delay + a slow first execution)
                nc.gpsimd.collective_compute(
                    "AllToAll", mybir.AluOpType.bypass,
                    replica_groups=[list(range(NCORES))],
                    ins=[warm_in.opt()], outs=[warm_out.opt()])

                # ---- per-chain emitters ----
                def emit_qk_chain(b, l, wsb, dst, tt4, xtq):
                    ps = ps_a.tile([128, 512], f32, name=f"qkps_{b}_{l}_{tt4}",
                                   tag="ps_a")
                    for kk in range(KT):
                        nc.tensor.matmul(
                            ps[:],
                            wsb[:, kk, 128 * l:128 * l + 128],
                            xtq[:, kk, :],
                            start=(kk == 0), stop=(kk == KT - 1))
                    nc.vector.tensor_copy(
                        dst[:, 512 * tt4:512 * tt4 + 512], ps[:])

                def emit_v_chain(b, tk, xtq):
                    s = tk % 4
                    ps = ps_a.tile([128, 512], f32, name=f"vps_{b}_{tk}",
                                   tag="ps_a")
                    for kk in range(KT):
                        nc.tensor.matmul(
                            ps[:, 0:HPC * D],
                            xtq[:, kk, 128 * s:128 * s + 128],
                            wv_sb[:, kk, :],
                            start=(kk == 0), stop=(kk == KT - 1))
                    v_ = v_pool.tile([128, HPC * D], bf, name=f"v_{b}_{tk}",
                                     tag="v")
                    nc.vector.tensor_copy(v_[:], ps[:, 0:HPC * D])
                    return v_

                def emit_outproj_chain(b, ff, y2):
                    ps = ps_a.tile([128, 512], f32, name=f"ops_{b}_{ff}",
                                   tag="ps_a")
                    for kk in range(KT):
                        nc.tensor.matmul(
                            ps[:, 0:256],
                            wp_sb[:, kk, 128 * ff:128 * ff + 128],
                            y2[:, kk, :],
                            start=(kk == 0), stop=(kk == KT - 1))
                    ob = ob_pool.tile([128, 256], f32, name=f"ob_{b}_{ff}",
                                      tag="ob")
                    nc.scalar.copy(ob[:], ps[:, 0:256])
                    nc.sync.dma_start(
                        out_d.ap()[128 * ff:128 * ff + 128,
                                   256 * b:256 * b + 256], ob[:])

                # state carried across batches
                qraw = {}   # (l, 'q'/'k') -> current batch [128, T] tile
                vt = {}     # (b, tk) -> v tile [128 t, 256 d]
                filler = []          # list of closures for PE filler work

                a2a_in_v = [a2a_in[b].rearrange(
                    "(blk row) t -> row blk t", blk=8) for b in range(B - 1)]
                a2a_out_v = [a2a_out[b].rearrange(
                    "(kk p) t -> p kk t", kk=KT) for b in range(B - 1)]
                a2a3_in_v = [a2a3_in[l].rearrange(
                    "(blk row) t -> row blk t", blk=8) for l in range(HPC)]
                a2a3_out_v = [a2a3_out[l].rearrange(
                    "(kk2 p) t -> p kk2 t", kk2=8) for l in range(HPC)]

                for b in range(B):
                    # ---------- pre-attention: QKV + RoPE ----------
                    for l in range(HPC):
                        for nm in ('q', 'k'):
                            qraw[(l, nm)] = qk_pool.tile(
                                [128, T], bf, name=f"{nm}raw_{b}_{l}",
                                tag="qk")
                    for tt4 in range(4):
                        xtq = emit_xtq(b, tt4)
                        for l in range(HPC):
                            emit_qk_chain(b, l, wq_sb, qraw[(l, 'q')], tt4,
                                          xtq)
                            emit_qk_chain(b, l, wk_sb, qraw[(l, 'k')], tt4,
                                          xtq)
                        for s in range(4):
                            tk = 4 * tt4 + s
                            if (b, tk) not in vt:
                                vt[(b, tk)] = emit_v_chain(b, tk, xtq)
                    if not wp_loaded[0]:
                        nc.sync.dma_start(
                            wp_sb[:],
                            wp_d.ap().rearrange("(kk p) f -> p kk f", kk=KT))
                        wp_loaded[0] = True

                    # RoPE in-place: raw = raw*cos + swap(raw)*sin
                    for l in range(HPC):
                        for nm in ('q', 'k'):
                            raw = qraw[(l, nm)]
                            sw = sw_pool.tile([128, T], bf,
                                              name=f"sw_{b}_{l}_{nm}",
                                              tag="sw")
                            nc.sync.dma_start(sw[0:64, :], raw[64:128, :])
                            nc.sync.dma_start(sw[64:128, :], raw[0:64, :])
                            nc.vector.tensor_tensor(
                                raw[:], raw[:], cos_sb[:], op=mult)
                            nc.vector.tensor_tensor(
                                sw[:], sw[:], sin_sb[:], op=mult)
                            nc.vector.tensor_tensor(
                                raw[:], raw[:], sw[:], op=add)

                    # PE filler work for the attention phase, in pop order:
                    # first v-chains of batch b+1 (no collective dependency
                    # -> can never stall the in-order PE queue), then the
                    # out-proj of batch b-1 (its y2 lands mid-attention at
                    # the latest, even when the collective runs slow).
                    filler = []

                    def mk_vfill(bn, tk):
                        def go():
                            vt[(bn, tk)] = emit_v_chain(
                                bn, tk, emit_xtq(bn, tk // 4))
                        return go

                    if b + 1 < B:
                        # prefetch next batch's first two x-quarters
                        emit_xtq(b + 1, 0)
                        emit_xtq(b + 1, 1)
                        filler += [("safe", mk_vfill(b + 1, tk))
                                   for tk in range(8)]
                    if b >= 1:
                        # y2 via the scalar DMA queue so it is not
                        # head-of-line blocked behind rope swaps / x loads
                        y2 = y2_pool.tile([128, KT, 256], bf,
                                          name=f"y2_{b - 1}", tag="y2")
                        nc.scalar.dma_start(y2[:], a2a_out_v[b - 1])
                        # for b==3 these are safe immediately (y2(2) has
                        # had a full batch period to arrive)
                        kind = "safe" if b == B - 1 else "y2"
                        filler += [
                            (kind, (lambda bb, f, yy: lambda:
                                    emit_outproj_chain(bb, f, yy))(
                                b - 1, ff, y2))
                            for ff in range(KT)]
                    # hold back ~8 chains to cover the post-attention
                    # collective-completion window of the last batch
                    pop_budget = [len(filler) - 8 if b == B - 1
                                  else len(filler)]

                    # ---------- attention (l-outer, jj-inner) ----------
                    fill_credit = [0.0]

                    def emit_av(l, jj, g, ex, yps, ng):
                        for i in range(2):
                            tk = 2 * g + i
                            nc.tensor.matmul(
                                yps[:],
                                vt[(b, tk)][:, 128 * l:128 * l + 128],
                                ex[:, 512 * i:512 * i + 512],
                                start=(g == 0 and i == 0),
                                stop=(g == ng - 1 and i == 1))

                    def pop_filler(allow_y2):
                        # out-proj(b-1) chains ("y2") may only enter the
                        # in-order PE queue during the second head's
                        # attention, when their y2 is certain to have
                        # arrived even if the collective ran very slow
                        if (filler and pop_budget[0] > 0
                                and (allow_y2 or filler[0][0] == "safe")):
                            pop_budget[0] -= 1
                            filler.pop(0)[1]()

                    for l in range(HPC):
                        for jj in range(4):
                            qrot = qraw[(l, 'q')]
                            krot = qraw[(l, 'k')]
                            ng = 2 * (jj + 1)
                            yps = ps_y.tile([128, 512], f32,
                                            name=f"yps_{b}_{l}_{jj}",
                                            tag="ps_y")
                            acc2 = acc_pool.tile([128, 1024], bf,
                                                 name=f"acc_{b}_{l}_{jj}",
                                                 tag="acc")
                            prev = None
                            for g in range(ng):
                                scps = ps_sc.tile(
                                    [128, 1024], f32,
                                    name=f"sc_{b}_{l}_{jj}_{g}", tag="ps_sc")
                                for i in range(2):
                                    tk = 2 * g + i
                                    nc.tensor.matmul(
                                        scps[:, 512 * i:512 * i + 512],
                                        krot[:, 128 * tk:128 * tk + 128],
                                        qrot[:, 512 * jj:512 * jj + 512],
                                        start=True, stop=True)
                                ex = ex_pool.tile(
                                    [128, 1024], bf,
                                    name=f"ex_{b}_{l}_{jj}_{g}", tag="ex")
                                nc.scalar.activation(
                                    ex[:], scps[:], Exp, scale=SCALE)
                                if g >= 2 * jj:   # diagonal pair: mask
                                    p = g - 2 * jj
                                    nc.vector.tensor_tensor(
                                        ex[:], ex[:],
                                        mask_sb[:, 1024 * p:1024 * p + 1024],
                                        op=mult)
                                if g == 0:
                                    nc.vector.tensor_copy(acc2[:], ex[:])
                                else:
                                    nc.vector.tensor_tensor(
                                        acc2[:], acc2[:], ex[:], op=add)
                                # software pipeline: AV lags scores by one
                                if prev is not None:
                                    emit_av(l, jj, prev[0], prev[1], yps, ng)
                                prev = (g, ex)
                                fill_credit[0] += 1.0
                                if fill_credit[0] >= 2.0:
                                    fill_credit[0] -= 2.0
                                    pop_filler(l == 1)
                            emit_av(l, jj, prev[0], prev[1], yps, ng)

                            # normalization, off the PE critical path
                            accf = accf_pool.tile([128, 512], bf,
                                                  name=f"af_{b}_{l}_{jj}",
                                                  tag="accf")
                            nc.vector.tensor_tensor(
                                accf[:], acc2[:, 0:512], acc2[:, 512:1024],
                                op=add)
                            pop_filler(l == 1)
                            sums = ps_a.tile([128, 512], f32,
                                             name=f"sums_{b}_{l}_{jj}",
                                             tag="ps_a")
                            nc.tensor.matmul(sums[:], ones_sb[:], accf[:],
                                             start=True, stop=True)
                            rec = rec_pool.tile([128, 512], f32,
                                                name=f"rec_{b}_{l}_{jj}",
                                                tag="rec")
                            nc.vector.reciprocal_approx_fast(rec[:], sums[:])
                            yn = yn_pool.tile([128, 512], bf,
                                              name=f"yn_{b}_{l}_{jj}",
                                              tag="yn")
                            nc.vector.tensor_tensor(
                                yn[:], yps[:], rec[:], op=mult)
                            # scatter to a2a_in: q-range 512*jj covers the
                            # two 256-col t-blocks 2jj, 2jj+1
                            if b < B - 1:
                                nc.sync.dma_start(
                                    a2a_in_v[b][128 * l:128 * l + 128,
                                                2 * jj:2 * jj + 2, :],
                                    yn[:].rearrange("p (i t) -> p i t", i=2))
                            else:
                                nc.sync.dma_start(
                                    a2a3_in_v[l][:, 2 * jj:2 * jj + 2, :],
                                    yn[:].rearrange("p (i t) -> p i t", i=2))
                        # batch 3: fire head l's half-A2A as soon as its
                        # attention is done, overlapping the other head
                        if b == B - 1:
                            nc.gpsimd.collective_compute(
                                "AllToAll", mybir.AluOpType.bypass,
                                replica_groups=[list(range(NCORES))],
                                ins=[a2a3_in[l].opt()],
                                outs=[a2a3_out[l].opt()])


                    # ---------- collective for this batch ----------
                    if b < B - 1:
                        nc.gpsimd.collective_compute(
                            "AllToAll",
                            mybir.AluOpType.bypass,
                            replica_groups=[list(range(NCORES))],
                            ins=[a2a_in[b].opt()],
                            outs=[a2a_out[b].opt()],
                        )
                    # leftover fillers: emit now (for the last batch these
                    # are the held-back chains covering the collective wait)
                    while filler:
                        filler.pop(0)[1]()

                # ---------- drain: out-proj for last batch ----------
                # head l's half-A2A delivers the even (l=0) / odd (l=1) kk
                # chunks of y^T. Pass A (even kk, complete per-ff chains)
                # overlaps the second collective; partials park as bf16 in
                # the now-dead qk tiles. Pass B adds the odd kk.
                y2h0 = y2_pool.tile([128, 8, 256], bf, name="y23_0",
                                    tag="y2")
                nc.scalar.dma_start(y2h0[:], a2a3_out_v[0])
                y2h1 = y2_pool.tile([128, 8, 256], bf, name="y23_1",
                                    tag="y2")
                nc.scalar.dma_start(y2h1[:], a2a3_out_v[1])
                part = [qk_pool.tile([128, T], bf, name=f"part_{i}",
                                     tag="qk") for i in range(2)]
                for ff in range(KT):          # pass A: even kk
                    ps = ps_a.tile([128, 512], f32, name=f"dpsA_{ff}",
                                   tag="ps_a")
                    for i8 in range(8):
                        nc.tensor.matmul(
                            ps[:, 0:256],
                            wp_sb[:, 2 * i8, 128 * ff:128 * ff + 128],
                            y2h0[:, i8, :],
                            start=(i8 == 0), stop=(i8 == 7))
                    nc.vector.tensor_copy(
                        part[ff // 8][:, 256 * (ff % 8):256 * (ff % 8) + 256],
                        ps[:, 0:256])
                for ff in range(KT):          # pass B: odd kk + partial
                    ps = ps_a.tile([128, 512], f32, name=f"dpsB_{ff}",
                                   tag="ps_a")
                    for i8 in range(8):
                        nc.tensor.matmul(
                            ps[:, 0:256],
                            wp_sb[:, 2 * i8 + 1, 128 * ff:128 * ff + 128],
                            y2h1[:, i8, :],
                            start=(i8 == 0), stop=(i8 == 7))
                    ob = ob_pool.tile([128, 256], f32, name=f"ob3_{ff}",
                                      tag="ob")
                    nc.vector.tensor_tensor(
                        ob[:], ps[:, 0:256],
                        part[ff // 8][:, 256 * (ff % 8):256 * (ff % 8) + 256],
                        op=add)
                    nc.sync.dma_start(
                        out_d.ap()[128 * ff:128 * ff + 128,
                                   256 * (B - 1):256 * (B - 1) + 256],
                        ob[:])

    nc.compile()
    return nc


def _prep_inputs(x, rope_freqs, W_attn, W_proj):
    x = np.asarray(x, np.float32)
    rope_freqs = np.asarray(rope_freqs, np.float32)
    W_attn = np.asarray(W_attn, np.float32)
    W_proj = np.asarray(W_proj, np.float32)

    xT = np.ascontiguousarray(x.reshape(BT, C).T).astype(BF16)
    perm = np.concatenate([np.arange(0, D, 2), np.arange(1, D, 2)])
    theta = np.outer(rope_freqs.astype(np.float64), np.arange(T))
    cos_, sin_ = np.cos(theta), np.sin(theta)
    ccT = np.concatenate([cos_, cos_], axis=0).astype(BF16)   # (128, T)
    ssT = np.concatenate([-sin_, sin_], axis=0).astype(BF16)  # (128, T)
    masks = np.zeros((128, 2048), np.float32)
    for i in range(4):
        masks[:, 512 * i:512 * (i + 1)] = (
            np.arange(512)[None, :] >= (np.arange(128)[:, None] + 128 * i))
    masks = masks.astype(BF16)
    wpT = np.ascontiguousarray(W_proj.T).astype(BF16)

    in_maps = []
    for r in range(NCORES):
        wq_rows, wk_rows, wv_rows = [], [], []
        for l in range(HPC):
            h = HPC * r + l
            wq_rows.append(W_attn[D * h:D * h + D][perm])
            wk_rows.append(W_attn[C + D * h:C + D * h + D][perm])
            wv_rows.append(W_attn[2 * C + D * h:2 * C + D * h + D])
        in_maps.append({
            "xT": xT,
            "wqT": np.ascontiguousarray(
                np.concatenate(wq_rows, 0).T).astype(BF16),
            "wkT": np.ascontiguousarray(
                np.concatenate(wk_rows, 0).T).astype(BF16),
            "wvT": np.ascontiguousarray(
                np.concatenate(wv_rows, 0).T).astype(BF16),
            "ccT": ccT,
            "ssT": ssT,
            "masks": masks,
            "wpT": wpT,
        })
    return in_maps


def _ensure_trace_support():
    """Register the axon NTFF profiling hook if the image's antenv lacks it,
    and stub out the artifact upload (no bucket access in-container)."""
    import types
    import sys as _sys
    import antenv

    if "antenv.axon_hooks" not in _sys.modules:
        try:
            import antenv.axon_hooks  # noqa: F401
        except ImportError:
            mod = types.ModuleType("antenv.axon_hooks")
            _holder = {}
            mod.set_axon_ntff_profile_hook = (
                lambda h: _holder.__setitem__("h", h))
            mod.get_axon_ntff_profile_hook = lambda: _holder.get("h")
            _sys.modules["antenv.axon_hooks"] = mod
            antenv.axon_hooks = mod
    import antenv.axon_hooks as ah

    if ah.get_axon_ntff_profile_hook() is None:
        try:
            from trn_agent_boot.trn_boot import _ntff_profile_via_ctypes
            hook = _ntff_profile_via_ctypes("/opt/axon/libaxon_pjrt.so")
            if hook is not None:
                ah.set_axon_ntff_profile_hook(hook)
        except Exception as e:  # profiling stays off; run still works
            print(f"ntff hook registration failed: {e}", file=sys.stderr)
    from concourse import bass_utils as bu
    bu.upload_artifacts = lambda tmpdir: f"local://{tmpdir}"


def kernel(x, rope_freqs, W_attn, W_proj):
    global LAST_EXEC_NS
    from concourse import bass_utils

    if "nc" not in _CACHE:
        _CACHE["nc"] = _build_nc()
    nc = _CACHE["nc"]

    in_maps = _prep_inputs(x, rope_freqs, W_attn, W_proj)
    trace = os.environ.get("KERNEL_TRACE", "0") == "1"
    tmpdir = None
    if trace:
        _ensure_trace_support()
        tmpdir = os.environ.get("KERNEL_TRACE_DIR") or None
    res = bass_utils.run_bass_kernel_spmd(
        nc, in_maps, core_ids=list(range(NCORES)), trace=trace,
        tmpdir=tmpdir)
    LAST_EXEC_NS = res.exec_time_ns

    # core r's outT: [2048 chan, 4 batches x 256 t]; batch b chunk holds
    # global rows 2048*b + 256*r ... + 256
    out = np.empty((BT, C), np.float32)
    for r in range(NCORES):
        outT = np.asarray(res.results[r]["outT"], np.float32)
        for b in range(B):
            out[2048 * b + 256 * r:2048 * b + 256 * r + 256, :] = \
                outT[:, 256 * b:256 * b + 256].T
    return np.ascontiguousarray(out).reshape(B, T, C)


# revision 36
# speedup vs baseline: 1.1047x; 1.0352x over previous
"""Causal self-attention (B=4,T=2048,C=2048,H=16,D=128) on 8 TRN2 NeuronCores.

Strategy: tensor-parallel over heads (2 heads/core) for QKV + attention.
Per-batch AllToAll redistributes y^T so each core holds all channels for
256 (b,t) rows per batch; the t-sharded output projection for batch b-1 is
interleaved as PE filler into batch b's attention groups, hiding both the
collective and the projection. bf16 matmuls, fp32 PSUM accumulation,
softmax without max-subtraction, causal masking via 0/1 tiles multiplied
post-exp on DVE, RoPE in-place via host-side weight-row permutation +
partition swap. All bulk loads are single multi-dim DMAs (AP.rearrange).
"""
import os
import sys

sys.path.insert(0, "/opt/trn_rl_repo")

import numpy as np
import ml_dtypes

B, T, C, H, D = 4, 2048, 2048, 16, 128
NCORES = 8
HPC = H // NCORES          # 2 heads per core
BT = B * T                 # 8192
BTPC = BT // NCORES        # 1024 (b,t) rows per core for out-proj
KT = C // 128              # 16 contraction tiles
SCALE = 1.0 / float(np.sqrt(D))
BF16 = ml_dtypes.bfloat16

LAST_EXEC_NS = None
_CACHE = {}


def _build_nc():
    from contextlib import ExitStack
    from concourse import bacc, tile, mybir

    bf = mybir.dt.bfloat16
    f32 = mybir.dt.float32
    mult = mybir.AluOpType.mult
    add = mybir.AluOpType.add
    Exp = mybir.ActivationFunctionType.Exp

    nc = bacc.Bacc("TRN2", target_bir_lowering=False, debug=False,
                   num_devices=NCORES)

    xT_d = nc.dram_tensor("xT", [C, BT], bf, kind="ExternalInput")
    wq_d = nc.dram_tensor("wqT", [C, HPC * D], bf, kind="ExternalInput")
    wk_d = nc.dram_tensor("wkT", [C, HPC * D], bf, kind="ExternalInput")
    wv_d = nc.dram_tensor("wvT", [C, HPC * D], bf, kind="ExternalInput")
    cos_d = nc.dram_tensor("ccT", [D, T], bf, kind="ExternalInput")
    sin_d = nc.dram_tensor("ssT", [D, T], bf, kind="ExternalInput")
    mask_d = nc.dram_tensor("masks", [128, 2048], bf, kind="ExternalInput")
    wp_d = nc.dram_tensor("wpT", [C, C], bf, kind="ExternalInput")
    out_d = nc.dram_tensor("outT", [C, BTPC], f32, kind="ExternalOutput")

    with tile.TileContext(nc) as tc:
        with tc.tile_pool(name="dram", bufs=1, space="DRAM") as dram:
            # per-batch A2A: block j of in = my 2 heads' y^T for core j's
            # 256 t of this batch; block i of out = core i's heads for my t.
            # batch 3 is split by head (l) so the first half overlaps the
            # second half of its attention.
            a2a_in = [dram.tile([2048, 256], bf, name=f"a2a_in{b}")
                      for b in range(B - 1)]
            a2a_out = [dram.tile([2048, 256], bf, name=f"a2a_out{b}")
                       for b in range(B - 1)]
            a2a3_in = [dram.tile([1024, 256], bf, name=f"a2a3_in{l}")
                       for l in range(HPC)]
            a2a3_out = [dram.tile([1024, 256], bf, name=f"a2a3_out{l}")
                        for l in range(HPC)]
            warm_in = dram.tile([128, 16], bf, name="warm_in")
            warm_out = dram.tile([128, 16], bf, name="warm_out")

            with ExitStack() as ab:
                const = ab.enter_context(tc.tile_pool(name="const", bufs=1))
                wpool = ab.enter_context(tc.tile_pool(name="w", bufs=1))
                xtq_pool = ab.enter_context(tc.tile_pool(name="xtq", bufs=2))
                qk_pool = ab.enter_context(tc.tile_pool(name="qk", bufs=4))
                sw_pool = ab.enter_context(tc.tile_pool(name="sw", bufs=2))
                v_pool = ab.enter_context(tc.tile_pool(name="v", bufs=25))
                ex_pool = ab.enter_context(tc.tile_pool(name="ex", bufs=3))
                acc_pool = ab.enter_context(tc.tile_pool(name="acc", bufs=2))
                accf_pool = ab.enter_context(tc.tile_pool(name="accf", bufs=3))
                rec_pool = ab.enter_context(tc.tile_pool(name="rec", bufs=2))
                yn_pool = ab.enter_context(tc.tile_pool(name="yn", bufs=3))
                y2_pool = ab.enter_context(tc.tile_pool(name="y2", bufs=2))
                ob_pool = ab.enter_context(tc.tile_pool(name="ob", bufs=2))
                ps_a = ab.enter_context(
                    tc.tile_pool(name="ps_a", bufs=2, space="PSUM"))
                ps_sc = ab.enter_context(
                    tc.tile_pool(name="ps_sc", bufs=2, space="PSUM"))
                ps_y = ab.enter_context(
                    tc.tile_pool(name="ps_y", bufs=2, space="PSUM"))

                # ---- weights first (first chains need wq/wk + x), then
                # constants (needed only from RoPE onward) ----
                xT_v = xT_d.ap().rearrange("(kk p) t -> p kk t", kk=KT)
                wp_loaded = [False]
                xtq_tiles = {}       # (b, tt4) -> xtq tile

                def emit_xtq(b, tt4):
                    if (b, tt4) in xtq_tiles:
                        return xtq_tiles[(b, tt4)]
                    t_ = xtq_pool.tile([128, KT, 512], bf,
                                       name=f"xtq_{b}_{tt4}", tag="xtq")
                    c0 = 2048 * b + 512 * tt4
                    nc.sync.dma_start(t_[:], xT_v[:, :, c0:c0 + 512])
                    xtq_tiles[(b, tt4)] = t_
                    return t_

                # w layout [128, kk, cols]: partition p of chunk kk = row
                # 128*kk + p of the [C, cols] DRAM tensor.
                wq_sb = wpool.tile([128, KT, HPC * D], bf, name="wq_sb")
                nc.sync.dma_start(
                    wq_sb[:], wq_d.ap().rearrange("(kk p) f -> p kk f", kk=KT))
                emit_xtq(0, 0)
                wk_sb = wpool.tile([128, KT, HPC * D], bf, name="wk_sb")
                nc.sync.dma_start(
                    wk_sb[:], wk_d.ap().rearrange("(kk p) f -> p kk f", kk=KT))
                wv_sb = wpool.tile([128, KT, HPC * D], bf, name="wv_sb")
                nc.sync.dma_start(
                    wv_sb[:], wv_d.ap().rearrange("(kk p) f -> p kk f", kk=KT))
                cos_sb = const.tile([D, T], bf, name="cos_sb")
                nc.sync.dma_start(cos_sb[:], cos_d.ap())
                sin_sb = const.tile([D, T], bf, name="sin_sb")
                nc.sync.dma_start(sin_sb[:], sin_d.ap())
                mask_sb = const.tile([128, 2048], bf, name="mask_sb")
                nc.sync.dma_start(mask_sb[:], mask_d.ap())
                ones_sb = const.tile([128, 128], bf, name="ones_sb")
                nc.vector.memset(ones_sb[:], 1.0)
                wp_sb = wpool.tile([128, KT, C], bf, name="wp_sb")

                # warm up the collectives machinery (first A2A otherwise
                # pays ~11us trigger delay + a slow first execution)
                nc.gpsimd.collective_compute(
                    "AllToAll", mybir.AluOpType.bypass,
                    replica_groups=[list(range(NCORES))],
                    ins=[warm_in.opt()], outs=[warm_out.opt()])

                # ---- per-chain emitters ----
                def emit_qk_chain(b, l, wsb, dst, tt4, xtq):
                    ps = ps_a.tile([128, 512], f32, name=f"qkps_{b}_{l}_{tt4}",
                                   tag="ps_a")
                    for kk in range(KT):
                        nc.tensor.matmul(
                            ps[:],
                            wsb[:, kk, 128 * l:128 * l + 128],
                            xtq[:, kk, :],
                            start=(kk == 0), stop=(kk == KT - 1))
                    nc.vector.tensor_copy(
                        dst[:, 512 * tt4:512 * tt4 + 512], ps[:])

                def emit_v_chain(b, tk, xtq):
                    s = tk % 4
                    ps = ps_a.tile([128, 512], f32, name=f"vps_{b}_{tk}",
                                   tag="ps_a")
                    for kk in range(KT):
                        nc.tensor.matmul(
                            ps[:, 0:HPC * D],
                            xtq[:, kk, 128 * s:128 * s + 128],
                            wv_sb[:, kk, :],
                            start=(kk == 0), stop=(kk == KT - 1))
                    v_ = v_pool.tile([128, HPC * D], bf, name=f"v_{b}_{tk}",
                                     tag="v")
                    nc.vector.tensor_copy(v_[:], ps[:, 0:HPC * D])
                    return v_

                def emit_outproj_chain(b, ff, y2):
                    ps = ps_a.tile([128, 512], f32, name=f"ops_{b}_{ff}",
                                   tag="ps_a")
                    for kk in range(KT):
                        nc.tensor.matmul(
                            ps[:, 0:256],
                            wp_sb[:, kk, 128 * ff:128 * ff + 128],
                            y2[:, kk, :],
                            start=(kk == 0), stop=(kk == KT - 1))
                    ob = ob_pool.tile([128, 256], f32, name=f"ob_{b}_{ff}",
                                      tag="ob")
                    nc.scalar.copy(ob[:], ps[:, 0:256])
                    nc.sync.dma_start(
                        out_d.ap()[128 * ff:128 * ff + 128,
                                   256 * b:256 * b + 256], ob[:])

                # state carried across batches
                qraw = {}   # (l, 'q'/'k') -> current batch [128, T] tile
                vt = {}     # (b, tk) -> v tile [128 t, 256 d]
                filler = []          # list of closures for PE filler work

                a2a_in_v = [a2a_in[b].rearrange(
                    "(blk row) t -> row blk t", blk=8) for b in range(B - 1)]
                a2a_out_v = [a2a_out[b].rearrange(
                    "(kk p) t -> p kk t", kk=KT) for b in range(B - 1)]
                a2a3_in_v = [a2a3_in[l].rearrange(
                    "(blk row) t -> row blk t", blk=8) for l in range(HPC)]
                a2a3_out_v = [a2a3_out[l].rearrange(
                    "(kk2 p) t -> p kk2 t", kk2=8) for l in range(HPC)]

                for b in range(B):
                    # ---------- pre-attention: QKV + RoPE ----------
                    for l in range(HPC):
                        for nm in ('q', 'k'):
                            qraw[(l, nm)] = qk_pool.tile(
                                [128, T], bf, name=f"{nm}raw_{b}_{l}",
                                tag="qk")
                    for tt4 in range(4):
                        xtq = emit_xtq(b, tt4)
                        for l in range(HPC):
                            emit_qk_chain(b, l, wq_sb, qraw[(l, 'q')], tt4,
                                          xtq)
                            emit_qk_chain(b, l, wk_sb, qraw[(l, 'k')], tt4,
                                          xtq)
                        for s in range(4):
                            tk = 4 * tt4 + s
                            if (b, tk) not in vt:
                                vt[(b, tk)] = emit_v_chain(b, tk, xtq)
                    if not wp_loaded[0]:
                        nc.sync.dma_start(
                            wp_sb[:],
                            wp_d.ap().rearrange("(kk p) f -> p kk f", kk=KT))
                        wp_loaded[0] = True

                    # RoPE in-place: raw = raw*cos + swap(raw)*sin
                    for l in range(HPC):
                        for nm in ('q', 'k'):
                            raw = qraw[(l, nm)]
                            sw = sw_pool.tile([128, T], bf,
                                              name=f"sw_{b}_{l}_{nm}",
                                              tag="sw")
                            nc.sync.dma_start(sw[0:64, :], raw[64:128, :])
                            nc.sync.dma_start(sw[64:128, :], raw[0:64, :])
                            nc.vector.tensor_tensor(
                                raw[:], raw[:], cos_sb[:], op=mult)
                            nc.vector.tensor_tensor(
                                sw[:], sw[:], sin_sb[:], op=mult)
                            nc.vector.tensor_tensor(
                                raw[:], raw[:], sw[:], op=add)

                    # PE filler work for the attention phase, in pop order:
                    # first v-chains of batch b+1 (no collective dependency
                    # -> can never stall the in-order PE queue), then the
                    # out-proj of batch b-1 (its y2 lands mid-attention at
                    # the latest, even when the collective runs slow).
                    filler = []

                    def mk_vfill(bn, tk):
                        def go():
                            vt[(bn, tk)] = emit_v_chain(
                                bn, tk, emit_xtq(bn, tk // 4))
                        return go

                    if b + 1 < B:
                        # prefetch next batch's first two x-quarters
                        emit_xtq(b + 1, 0)
                        emit_xtq(b + 1, 1)
                        filler += [("safe", mk_vfill(b + 1, tk))
                                   for tk in range(8)]
                    if b >= 1:
                        # y2 via the scalar DMA queue so it is not
                        # head-of-line blocked behind rope swaps / x loads
                        y2 = y2_pool.tile([128, KT, 256], bf,
                                          name=f"y2_{b - 1}", tag="y2")
                        nc.scalar.dma_start(y2[:], a2a_out_v[b - 1])
                        # for b==3 these are safe immediately (y2(2) has
                        # had a full batch period to arrive)
                        kind = "safe" if b == B - 1 else "y2"
                        filler += [
                            (kind, (lambda bb, f, yy: lambda:
                                    emit_outproj_chain(bb, f, yy))(
                                b - 1, ff, y2))
                            for ff in range(KT)]
                    # hold back ~8 chains to cover the post-attention
                    # collective-completion window of the last batch
                    pop_budget = [len(filler) - 8 if b == B - 1
                                  else len(filler)]

                    # ---------- attention (l-outer, jj-inner) ----------
                    fill_credit = [0.0]

                    def emit_av(l, jj, g, ex, yps, ng):
                        for i in range(2):
                            tk = 2 * g + i
                            nc.tensor.matmul(
                                yps[:],
                                vt[(b, tk)][:, 128 * l:128 * l + 128],
                                ex[:, 512 * i:512 * i + 512],
                                start=(g == 0 and i == 0),
                                stop=(g == ng - 1 and i == 1))

                    def pop_filler(allow_y2):
                        # out-proj(b-1) chains ("y2") may only enter the
                        # in-order PE queue during the second head's
                        # attention, when their y2 is certain to have
                        # arrived even if the collective ran very slow
                        if (filler and pop_budget[0] > 0
                                and (allow_y2 or filler[0][0] == "safe")):
                            pop_budget[0] -= 1
                            filler.pop(0)[1]()

                    for l in range(HPC):
                        for jj in range(4):
                            qrot = qraw[(l, 'q')]
                            krot = qraw[(l, 'k')]
                            ng = 2 * (jj + 1)
                            yps = ps_y.tile([128, 512], f32,
                                            name=f"yps_{b}_{l}_{jj}",
                                            tag="ps_y")
                            acc2 = acc_pool.tile([128, 1024], bf,
                                                 name=f"acc_{b}_{l}_{jj}",
                                                 tag="acc")
                            prev = None
                            for g in range(ng):
                                scps = ps_sc.tile(
                                    [128, 1024], f32,
                                    name=f"sc_{b}_{l}_{jj}_{g}", tag="ps_sc")
                                for i in range(2):
                                    tk = 2 * g + i
                                    nc.tensor.matmul(
                                        scps[:, 512 * i:512 * i + 512],
                                        krot[:, 128 * tk:128 * tk + 128],
                                        qrot[:, 512 * jj:512 * jj + 512],
                                        start=True, stop=True)
                                ex = ex_pool.tile(
                                    [128, 1024], bf,
                                    name=f"ex_{b}_{l}_{jj}_{g}", tag="ex")
                                nc.scalar.activation(
                                    ex[:], scps[:], Exp, scale=SCALE)
                                if g >= 2 * jj:   # diagonal pair: mask
                                    p = g - 2 * jj
                                    nc.vector.tensor_tensor(
                                        ex[:], ex[:],
                                        mask_sb[:, 1024 * p:1024 * p + 1024],
                                        op=mult)
                                if g == 0:
                                    nc.vector.tensor_copy(acc2[:], ex[:])
                                else:
                                    nc.vector.tensor_tensor(
                                        acc2[:], acc2[:], ex[:], op=add)
                                # software pipeline: AV lags scores by one
                                if prev is not None:
                                    emit_av(l, jj, prev[0], prev[1], yps, ng)
                                prev = (g, ex)
                                fill_credit[0] += 1.0
                                if fill_credit[0] >= 2.0:
                                    fill_credit[0] -= 2.0
                                    pop_filler(l == 1)
                            emit_av(l, jj, prev[0], prev[1], yps, ng)

                            # normalization, off the PE critical path
                            accf = accf_pool.tile([128, 512], bf,
                                                  name=f"af_{b}_{l}_{jj}",
                                                  tag="accf")
                            nc.vector.tensor_tensor(
                                accf[:], acc2[:, 0:512], acc2[:, 512:1024],
                                op=add)
                            pop_filler(l == 1)
                            sums = ps_a.tile([128, 512], f32,
                                             name=f"sums_{b}_{l}_{jj}",
                                             tag="ps_a")
                            nc.tensor.matmul(sums[:], ones_sb[:], accf[:],
                                             start=True, stop=True)
                            rec = rec_pool.tile([128, 512], f32,
                                                name=f"rec_{b}_{l}_{jj}",
                                                tag="rec")
                            nc.vector.reciprocal_approx_fast(rec[:], sums[:])
                            yn = yn_pool.tile([128, 512], bf,
                                              name=f"yn_{b}_{l}_{jj}",
                                              tag="yn")
                            nc.vector.tensor_tensor(
                                yn[:], yps[:], rec[:], op=mult)
                            # scatter to a2a_in: q-range 512*jj covers the
                            # two 256-col t-blocks 2jj, 2jj+1
                            if b < B - 1:
                                nc.sync.dma_start(
                                    a2a_in_v[b][128 * l:128 * l + 128,
                                                2 * jj:2 * jj + 2, :],
                                    yn[:].rearrange("p (i t) -> p i t", i=2))
                            else:
                                nc.sync.dma_start(
                                    a2a3_in_v[l][:, 2 * jj:2 * jj + 2, :],
                                    yn[:].rearrange("p (i t) -> p i t", i=2))
                        # batch 3: fire head l's half-A2A as soon as its
                        # attention is done, overlapping the other head
                        if b == B - 1:
                            nc.gpsimd.collective_compute(
                                "AllToAll", mybir.AluOpType.bypass,
                                replica_groups=[list(range(NCORES))],
                                ins=[a2a3_in[l].opt()],
                                outs=[a2a3_out[l].opt()])


                    # ---------- collective for this batch ----------
                    if b < B - 1:
                        nc.gpsimd.collective_compute(
                            "AllToAll",
                            mybir.AluOpType.bypass,
                            replica_groups=[list(range(NCORES))],
                            ins=[a2a_in[b].opt()],
                            outs=[a2a_out[b].opt()],
                        )
                    # leftover fillers: emit now (for the last batch these
                    # are the held-back chains covering the collective wait)
                    while filler:
                        filler.pop(0)[1]()

                # ---------- drain: out-proj for last batch ----------
                # head l's half-A2A delivers the even (l=0) / odd (l=1) kk
                # chunks of y^T. Pass A (even kk, complete per-ff chains)
                # overlaps the second collective; partials park as bf16 in
                # the now-dead qk tiles. Pass B adds the odd kk.
                y2h0 = y2_pool.tile([128, 8, 256], bf, name="y23_0",
                                    tag="y2")
                nc.scalar.dma_start(y2h0[:], a2a3_out_v[0])
                y2h1 = y2_pool.tile([128, 8, 256], bf, name="y23_1",
                                    tag="y2")
                nc.scalar.dma_start(y2h1[:], a2a3_out_v[1])
                part = [qk_pool.tile([128, T], bf, name=f"part_{i}",
                                     tag="qk") for i in range(2)]
                for ff in range(KT):          # pass A: even kk
                    ps = ps_a.tile([128, 512], f32, name=f"dpsA_{ff}",
                                   tag="ps_a")
                    for i8 in range(8):
                        nc.tensor.matmul(
                            ps[:, 0:256],
                            wp_sb[:, 2 * i8, 128 * ff:128 * ff + 128],
                            y2h0[:, i8, :],
                            start=(i8 == 0), stop=(i8 == 7))
                    nc.vector.tensor_copy(
                        part[ff // 8][:, 256 * (ff % 8):256 * (ff % 8) + 256],
                        ps[:, 0:256])
                for ff in range(KT):          # pass B: odd kk + partial
                    ps = ps_a.tile([128, 512], f32, name=f"dpsB_{ff}",
                                   tag="ps_a")
                    for i8 in range(8):
                        nc.tensor.matmul(
                            ps[:, 0:256],
                            wp_sb[:, 2 * i8 + 1, 128 * ff:128 * ff + 128],
                            y2h1[:, i8, :],
                            start=(i8 == 0), stop=(i8 == 7))
                    ob = ob_pool.tile([128, 256], f32, name=f"ob3_{ff}",
                                      tag="ob")
                    nc.vector.tensor_tensor(
                        ob[:], ps[:, 0:256],
                        part[ff // 8][:, 256 * (ff % 8):256 * (ff % 8) + 256],
                        op=add)
                    nc.sync.dma_start(
                        out_d.ap()[128 * ff:128 * ff + 128,
                                   256 * (B - 1):256 * (B - 1) + 256],
                        ob[:])

    nc.compile()
    return nc


def _prep_inputs(x, rope_freqs, W_attn, W_proj):
    x = np.asarray(x, np.float32)
    rope_freqs = np.asarray(rope_freqs, np.float32)
    W_attn = np.asarray(W_attn, np.float32)
    W_proj = np.asarray(W_proj, np.float32)

    xT = np.ascontiguousarray(x.reshape(BT, C).T).astype(BF16)
    perm = np.concatenate([np.arange(0, D, 2), np.arange(1, D, 2)])
    theta = np.outer(rope_freqs.astype(np.float64), np.arange(T))
    cos_, sin_ = np.cos(theta), np.sin(theta)
    ccT = np.concatenate([cos_, cos_], axis=0).astype(BF16)   # (128, T)
    ssT = np.concatenate([-sin_, sin_], axis=0).astype(BF16)  # (128, T)
    masks = np.zeros((128, 2048), np.float32)
    for i in range(4):
        masks[:, 512 * i:512 * (i + 1)] = (
            np.arange(512)[None, :] >= (np.arange(128)[:, None] + 128 * i))
    masks = masks.astype(BF16)
    wpT = np.ascontiguousarray(W_proj.T).astype(BF16)

    in_maps = []
    for r in range(NCORES):
        wq_rows, wk_rows, wv_rows = [], [], []
        for l in range(HPC):
            h = HPC * r + l
            wq_rows.append(W_attn[D * h:D * h + D][perm])
            wk_rows.append(W_attn[C + D * h:C + D * h + D][perm])
            wv_rows.append(W_attn[2 * C + D * h:2 * C + D * h + D])
        in_maps.append({
            "xT": xT,
            "wqT": np.ascontiguousarray(
                np.concatenate(wq_rows, 0).T).astype(BF16),
            "wkT": np.ascontiguousarray(
                np.concatenate(wk_rows, 0).T).astype(BF16),
            "wvT": np.ascontiguousarray(
                np.concatenate(wv_rows, 0).T).astype(BF16),
            "ccT": ccT,
            "ssT": ssT,
            "masks": masks,
            "wpT": wpT,
        })
    return in_maps


def _ensure_trace_support():
    """Register the axon NTFF profiling hook if the image's antenv lacks it,
    and stub out the artifact upload (no bucket access in-container)."""
    import types
    import sys as _sys
    import antenv

    if "antenv.axon_hooks" not in _sys.modules:
        try:
            import antenv.axon_hooks  # noqa: F401
        except ImportError:
            mod = types.ModuleType("antenv.axon_hooks")
            _holder = {}
            mod.set_axon_ntff_profile_hook = (
                lambda h: _holder.__setitem__("h", h))
            mod.get_axon_ntff_profile_hook = lambda: _holder.get("h")
            _sys.modules["antenv.axon_hooks"] = mod
            antenv.axon_hooks = mod
    import antenv.axon_hooks as ah

    if ah.get_axon_ntff_profile_hook() is None:
        try:
            from trn_agent_boot.trn_boot import _ntff_profile_via_ctypes
            hook = _ntff_profile_via_ctypes("/opt/axon/libaxon_pjrt.so")
            if hook is not None:
                ah.set_axon_ntff_profile_hook(hook)
        except Exception as e:  # profiling stays off; run still works
            print(f"ntff hook registration failed: {e}", file=sys.stderr)
    from concourse import bass_utils as bu
    bu.upload_artifacts = lambda tmpdir: f"local://{tmpdir}"


def kernel(x, rope_freqs, W_attn, W_proj):
    global LAST_EXEC_NS
    from concourse import bass_utils

    if "nc" not in _CACHE:
        _CACHE["nc"] = _build_nc()
    nc = _CACHE["nc"]

    in_maps = _prep_inputs(x, rope_freqs, W_attn, W_proj)
    trace = os.environ.get("KERNEL_TRACE", "0") == "1"
    tmpdir = None
    if trace:
        _ensure_trace_support()
        tmpdir = os.environ.get("KERNEL_TRACE_DIR") or None
    res = bass_utils.run_bass_kernel_spmd(
        nc, in_maps, core_ids=list(range(NCORES)), trace=trace,
        tmpdir=tmpdir)
    LAST_EXEC_NS = res.exec_time_ns

    # core r's outT: [2048 chan, 4 batches x 256 t]; batch b chunk holds
    # global rows 2048*b + 256*r ... + 256
    out = np.empty((BT, C), np.float32)
    for r in range(NCORES):
        outT = np.asarray(res.results[r]["outT"], np.float32)
        for b in range(B):
            out[2048 * b + 256 * r:2048 * b + 256 * r + 256, :] = \
                outT[:, 256 * b:256 * b + 256].T
    return np.ascontiguousarray(out).reshape(B, T, C)
